# revision 1
# baseline (speedup 1.0000x reference)
"""ExpertGraphConv Trainium2 kernel.

Computation (per token n, experts E=16, D=512):
    adjacency = sigmoid(adj_logits)                       [E,E]
    a = x @ w1 ; c = x @ w2                               [N,E]
    gate[n,i,j] = adjacency[i,j]*sigmoid(a[n,i]+c[n,j]+b)*(1-eye)
    neighbor = einsum('nij,njd->nid', gate, x)
    out = gelu(neighbor @ Wn.T + x @ Ws.T + bn + bs)

Mapping (per core, data-parallel over tokens; core k takes batch k):
  rows = tokens*E = 8192 per core, processed in 64 blocks of 128 rows
  (8 tokens per block), grouped in 4-block superblocks.  Per block:
    - PE-transpose the x block (fp32r transpose w/ identity) to get xT
      (din on partitions); a-row GEMM (lhsT=w1) per superblock
    - c = x @ w2 on DVE (elementwise mult + free-dim reduce), a
      broadcast along partitions via GPSIMD partition_broadcast
    - sigmoid via tanh so the whole kernel stays in the single
      'gelu_and_others' ACT table set (sigmoid+gelu would thrash the
      ~2.7us ACT table switch): gate = blockdiag(0.5*adj^T, zero
      diag) * (tanh((a+c+b)/2) + 1)
    - xw = x @ Wn^T (psum->sbuf via ACT), h = x @ Ws^T + gate^T @ xw
      accumulated in one PSUM group; bias add on DVE; gelu on ACT
  Matmuls run as float32r (full-rate fp32 path on TRN2, ~270ns per
  [128x128]@[128x512] MM vs ~1100ns for plain fp32); rel err vs the
  fp32 reference is ~2e-4.  Avoid: small-N matmuls on PE (group
  overheads dominate), gpsimd 2-input elementwise (~4us/tile), and
  tensor_tensor_reduce (crashes TRN2 here).
"""

import sys

sys.path.insert(0, "/opt/trn_rl_repo")

import numpy as np

import concourse.bacc as bacc
import concourse.mybir as mybir
import concourse.tile as tile
from concourse.masks import make_identity

F32 = mybir.dt.float32
F32R = mybir.dt.float32r

B, S, E, D = 8, 512, 16, 512
N_CORES = 8
ROWS_PER_CORE = (B // N_CORES) * S * E  # 8192
KC = D // 128  # 4 contraction chunks

AF = mybir.ActivationFunctionType


def build_program(n_rows=ROWS_PER_CORE, repeat=1, use_f32r=True,
                  final_act=None, timing_io=False, ablate=""):
    """Build the per-core Bass program. Input x is the core's [n_rows, D]
    row-major shard; all small tensors are replicated.

    timing_io=True replaces the big x/out external tensors with internal
    DRAM (zero-filled on device) so per-call host I/O is tiny; used only
    for execution-time measurement."""
    assert n_rows % 512 == 0
    if final_act is None:
        final_act = AF.Gelu
    nc = bacc.Bacc("TRN2", target_bir_lowering=False, debug=False,
                   num_devices=N_CORES)

    if timing_io:
        x_d = nc.dram_tensor("x_int", [n_rows, D], F32).ap()
        out_d = nc.dram_tensor("out_int", [n_rows, D], F32).ap()
        marker_d = nc.dram_tensor("marker", [128, D], F32,
                                  kind="ExternalOutput").ap()
    else:
        x_d = nc.dram_tensor("x", [n_rows, D], F32, kind="ExternalInput").ap()
    wn_d = nc.dram_tensor("wn", [D, D], F32, kind="ExternalInput").ap()
    ws_d = nc.dram_tensor("ws", [D, D], F32, kind="ExternalInput").ap()
    mww_d = nc.dram_tensor("mww", [2 * D], F32, kind="ExternalInput").ap()
    bn_d = nc.dram_tensor("bn", [D], F32, kind="ExternalInput").ap()
    bs_d = nc.dram_tensor("bs", [D], F32, kind="ExternalInput").ap()
    mwb_d = nc.dram_tensor("mwb", [1, 1], F32, kind="ExternalInput").ap()
    adj_d = nc.dram_tensor("adj", [E, E], F32, kind="ExternalInput").ap()
    if not timing_io:
        out_d = nc.dram_tensor("out", [n_rows, D], F32,
                               kind="ExternalOutput").ap()

    MDT = F32R if use_f32r else F32

    with tile.TileContext(nc) as tc:
        from contextlib import ExitStack

        with ExitStack() as ctx:
            consts = ctx.enter_context(tc.tile_pool(name="consts", bufs=1))

            # ---- constants ----
            ident = consts.tile([128, 128], F32)
            make_identity(nc, ident)
            ident_r = consts.tile([128, 128], MDT)
            nc.vector.tensor_copy(ident_r[:], ident[:])

            # weights natural [dout, din] -> [128, 4, D] (p=dout%128)
            wn_nat = consts.tile([128, KC, D], F32)
            ws_nat = consts.tile([128, KC, D], F32)
            nc.sync.dma_start(wn_nat[:], wn_d.rearrange("(o p) f -> p o f", p=128))
            nc.sync.dma_start(ws_nat[:], ws_d.rearrange("(o p) f -> p o f", p=128))

            # transposed weights W^T [din(p), chunk, dout]
            wnT = consts.tile([128, KC, D], MDT)
            wsT = consts.tile([128, KC, D], MDT)
            with tc.tile_pool(name="wps", bufs=2, space="PSUM") as wps:
                for nat, wT in ((wn_nat, wnT), (ws_nat, wsT)):
                    for k in range(KC):  # din chunk
                        ps = wps.tile([128, D], F32, tag="wps")
                        for j in range(KC):  # dout chunk
                            nc.tensor.transpose(
                                ps[:, j * 128:(j + 1) * 128],
                                nat[:, j, k * 128:(k + 1) * 128],
                                ident,
                            )
                        nc.vector.tensor_copy(wT[:, k, :], ps[:])

            # w12 [din(p), chunk, 2]
            w12f = consts.tile([128, KC, 2], F32)
            for c in range(2):
                nc.sync.dma_start(
                    w12f[:, :, c],
                    mww_d[c * D:(c + 1) * D].rearrange("(o p) -> p o", p=128))
            w12 = consts.tile([128, KC, 2], MDT)
            nc.vector.tensor_copy(w12[:], w12f[:])

            # bias tile bn+bs broadcast to all partitions [128, D]
            btmp = consts.tile([1, D], F32)
            btmp2 = consts.tile([1, D], F32)
            nc.sync.dma_start(btmp[:], bn_d[None, :])
            nc.sync.dma_start(btmp2[:], bs_d[None, :])
            nc.vector.tensor_tensor(btmp[:], btmp[:], btmp2[:],
                                    mybir.AluOpType.add)
            bias_tile = consts.tile([128, D], F32)
            nc.gpsimd.partition_broadcast(bias_tile[:], btmp[:])

            # w2 broadcast to all partitions (c = x @ w2 computed on DVE)
            w2row = consts.tile([1, D], F32)
            nc.sync.dma_start(w2row[:], mww_d[None, D:2 * D])
            w2bc = consts.tile([128, D], F32)
            nc.gpsimd.partition_broadcast(w2bc[:], w2row[:])

            mwb = consts.tile([1, 1], F32)
            nc.sync.dma_start(mwb[:], mwb_d[:])
            # b/2 replicated to all partitions, for the per-partition tanh bias
            bhalf = consts.tile([128, 1], F32)
            nc.gpsimd.partition_broadcast(bhalf[:], mwb[:])
            nc.scalar.mul(bhalf[:], bhalf[:], 0.5)

            # A16[j,i] = 0.25*(tanh(adj_logits[i,j]/2)+1) = 0.5*sigmoid(adjL)^T,
            # diag zeroed.  Abd = 8x block-diagonal replication.
            adjT = consts.tile([E, E], F32)
            with nc.allow_non_contiguous_dma(reason="one-time 16x16 transpose load"):
                nc.sync.dma_start(adjT[:], adj_d.rearrange("i j -> j i"))
            a16 = consts.tile([E, E], F32)
            nc.scalar.activation(a16[:], adjT[:], AF.Tanh, scale=0.5)
            nc.vector.tensor_scalar(a16[:], a16[:], 1.0, 0.25,
                                    mybir.AluOpType.add, mybir.AluOpType.mult)
            nc.gpsimd.affine_select(
                out=a16, in_=a16, compare_op=mybir.AluOpType.not_equal,
                fill=0.0, base=0, pattern=[[-1, E]], channel_multiplier=1)
            abd = consts.tile([128, 128], F32)
            nc.gpsimd.memset(abd[:], 0.0)
            for t in range(8):
                nc.sync.dma_start(
                    abd[t * E:(t + 1) * E, t * E:(t + 1) * E], a16[:])
            abd_r = consts.tile([128, 128], MDT)
            nc.vector.tensor_copy(abd_r[:], abd[:])
            xt_dummy = consts.tile([128, KC, 512], MDT)
            nc.vector.tensor_copy(xt_dummy[:, 0, :], abd[:, 0:1].to_broadcast((128, 512)))
            nc.vector.tensor_copy(xt_dummy[:, 1, :], xt_dummy[:, 0, :])
            nc.vector.tensor_copy(xt_dummy[:, 2, :], xt_dummy[:, 0, :])
            nc.vector.tensor_copy(xt_dummy[:, 3, :], xt_dummy[:, 0, :])

            if timing_io:
                # zero-fill the internal x so timing runs on defined data
                zt = consts.tile([128, D], F32)
                nc.gpsimd.memset(zt[:], 0.0)
                for blk in range(n_rows // 128):
                    nc.sync.dma_start(
                        x_d[blk * 128:(blk + 1) * 128, :], zt[:])

            # ---- main loop pools ----
            p_xn = ctx.enter_context(tc.tile_pool(name="p_xn", bufs=8))
            p_xt = ctx.enter_context(tc.tile_pool(name="p_xt", bufs=2))
            p_lr = ctx.enter_context(tc.tile_pool(name="p_lr", bufs=3))
            p_g = ctx.enter_context(tc.tile_pool(name="p_g", bufs=3))
            p_xw = ctx.enter_context(tc.tile_pool(name="p_xw", bufs=2))
            p_o = ctx.enter_context(tc.tile_pool(name="p_o", bufs=3))
            ps_t = ctx.enter_context(tc.tile_pool(name="ps_t", bufs=2, space="PSUM"))
            ps_ac = ctx.enter_context(tc.tile_pool(name="ps_ac", bufs=1, space="PSUM"))
            ps_xw = ctx.enter_context(tc.tile_pool(name="ps_xw", bufs=2, space="PSUM"))
            ps_h = ctx.enter_context(tc.tile_pool(name="ps_h", bufs=3, space="PSUM"))

            def emit_block_load(sb, b2, xt_tile, xn_list):
                blk = sb * 4 + b2
                xn = p_xn.tile([128, D], MDT, tag="xn")
                xn_list.append(xn)
                nc.sync.dma_start(
                    xn[:], x_d[blk * 128:(blk + 1) * 128, :].bitcast(MDT))
                if "notrans" in ablate:
                    return
                pt = ps_t.tile([128, D], MDT, tag="pt")
                for k in range(KC):
                    nc.tensor.transpose(
                        pt[:, k * 128:(k + 1) * 128],
                        xn[:, k * 128:(k + 1) * 128], ident_r)
                ptv = pt.rearrange("p (o f) -> p o f", o=KC)
                nc.vector.tensor_copy(
                    xt_tile[:, 0:2, b2 * 128:(b2 + 1) * 128], ptv[:, 0:2, :])
                nc.scalar.copy(
                    xt_tile[:, 2:4, b2 * 128:(b2 + 1) * 128], ptv[:, 2:4, :])

            def body(_iv=None):
                NSB = n_rows // 512
                # prologue: first superblock's loads + transposes
                xt_cur = p_xt.tile([128, KC, 512], MDT, tag="xt")
                xn_cur = []
                for b2 in range(4):
                    emit_block_load(0, b2, xt_cur, xn_cur)
                for sb in range(NSB):
                    xt = xt_cur if "notrans" not in ablate else xt_dummy
                    # a GEMM over all 4 blocks: [1, 512] row form
                    pac = ps_ac.tile([1, 512], F32, tag="pac")
                    for k in range(KC):
                        nc.tensor.matmul(
                            pac[:], w12[:, k, 0:1], xt[:, k, :],
                            start=(k == 0), stop=(k == KC - 1))
                    a_row = p_lr.tile([1, 512], F32, tag="a_row")
                    nc.scalar.copy(a_row[:], pac[:])

                    if sb + 1 < NSB:
                        xt_next = p_xt.tile([128, KC, 512], MDT, tag="xt")
                        xn_next = []
                    for b2 in range(4):
                        blk = sb * 4 + b2
                        bsl = slice(b2 * 128, (b2 + 1) * 128)

                        # xw = x @ Wn^T
                        if "noneighbor" not in ablate:
                            pxw = ps_xw.tile([128, D], F32, tag="pxw")
                            for k in range(KC):
                                nc.tensor.matmul(
                                    pxw[:], xt[:, k, bsl], wnT[:, k, :],
                                    start=(k == 0), stop=(k == KC - 1))

                        if "nogate" in ablate:
                            gate = abd_r
                        else:
                            # cb = 0.5*(x @ w2) + b/2 off the PE (DVE)
                            scr = p_lr.tile([128, D], F32, tag="scr")
                            cred = p_lr.tile([128, 1], F32, tag="cred")
                            cb = p_lr.tile([128, 1], F32, tag="cb")
                            nc.vector.tensor_tensor(
                                scr[:], xn_cur[b2][:].bitcast(F32), w2bc[:],
                                mybir.AluOpType.mult)
                            nc.vector.tensor_reduce(
                                cred[:], scr[:], mybir.AxisListType.X,
                                mybir.AluOpType.add)
                            nc.vector.tensor_scalar(cb[:], cred[:], 0.5,
                                                    bhalf[:],
                                                    mybir.AluOpType.mult,
                                                    mybir.AluOpType.add)

                            # t[j,i] = tanh((a[i] + c[j] + b)/2);  a bcast
                            # along partitions, c+b as per-partition bias
                            a_bc = p_g.tile([128, 128], F32, tag="a_bc")
                            nc.gpsimd.partition_broadcast(a_bc[:],
                                                          a_row[0:1, bsl])
                            tt = p_g.tile([128, 128], F32, tag="tt")
                            gate = p_g.tile([128, 128], MDT, tag="gate")
                            nc.scalar.activation(tt[:], a_bc[:], AF.Tanh,
                                                 scale=0.5, bias=cb[:])
                            nc.vector.tensor_scalar(gate[:], tt[:], 1.0, None,
                                                    mybir.AluOpType.add)
                            nc.vector.tensor_tensor(gate[:], gate[:], abd[:],
                                                    mybir.AluOpType.mult)

                        # h = x@Ws^T + gate^T @ xw
                        ph = ps_h.tile([128, D], F32, tag="ph")
                        if "noneighbor" in ablate:
                            for k in range(KC):
                                nc.tensor.matmul(
                                    ph[:], xt[:, k, bsl], wsT[:, k, :],
                                    start=(k == 0), stop=(k == KC - 1))
                        else:
                            xw = p_xw.tile([128, D], MDT, tag="xw")
                            nc.scalar.copy(xw[:], pxw[:])
                            for k in range(KC):
                                nc.tensor.matmul(
                                    ph[:], xt[:, k, bsl], wsT[:, k, :],
                                    start=(k == 0), stop=False)
                            # next superblock's transposes interleaved here:
                            # extra PE time for the gate chain + xw copy to
                            # land before the neighbor matmul consumes them
                            if sb + 1 < NSB:
                                emit_block_load(sb + 1, b2, xt_next, xn_next)
                            nc.tensor.matmul(ph[:], gate[:], xw[:],
                                             start=False, stop=True)

                        # out = gelu(h + bias)
                        ot = p_o.tile([128, D], F32, tag="ot")
                        nc.vector.tensor_tensor(ot[:], ph[:], bias_tile[:],
                                                mybir.AluOpType.add)
                        nc.scalar.activation(ot[:], ot[:], final_act)
                        nc.sync.dma_start(
                            out_d[blk * 128:(blk + 1) * 128, :], ot[:])
                        if timing_io and blk == n_rows // 128 - 1:
                            nc.sync.dma_start(marker_d[:], ot[:])
                    if sb + 1 < NSB:
                        xt_cur = xt_next
                        xn_cur = xn_next

            if repeat == 1:
                body()
            else:
                with tc.For_i(0, repeat, 1):
                    body()

    nc.compile()
    return nc


_PROGRAMS = {}


def _get_program(n_rows=ROWS_PER_CORE, repeat=1, use_f32r=True,
                 timing_io=False, ablate=""):
    key = (n_rows, repeat, use_f32r, timing_io, ablate)
    if key not in _PROGRAMS:
        _PROGRAMS[key] = build_program(n_rows, repeat, use_f32r,
                                       timing_io=timing_io, ablate=ablate)
    return _PROGRAMS[key]


def make_in_maps(expert_features, Wn, bn, Ws, bs, mw_w, mw_b, adj_logits,
                 n_cores=N_CORES):
    x = np.ascontiguousarray(np.asarray(expert_features, dtype=np.float32))
    x = x.reshape(B * S * E, D)
    rows = x.shape[0] // n_cores
    common = {
        "wn": np.ascontiguousarray(np.asarray(Wn, dtype=np.float32)),
        "ws": np.ascontiguousarray(np.asarray(Ws, dtype=np.float32)),
        "mww": np.ascontiguousarray(np.asarray(mw_w, dtype=np.float32)),
        "bn": np.ascontiguousarray(np.asarray(bn, dtype=np.float32)),
        "bs": np.ascontiguousarray(np.asarray(bs, dtype=np.float32)),
        "mwb": np.asarray(mw_b, dtype=np.float32).reshape(1, 1),
        "adj": np.ascontiguousarray(np.asarray(adj_logits, dtype=np.float32)),
    }
    return [
        {"x": np.ascontiguousarray(x[k * rows:(k + 1) * rows]), **common}
        for k in range(n_cores)
    ]


def kernel(expert_features, Wn, bn, Ws, bs, mw_w, mw_b, adj_logits):
    from concourse.bass_utils import run_bass_kernel_spmd

    nc = _get_program()
    in_maps = make_in_maps(expert_features, Wn, bn, Ws, bs, mw_w, mw_b,
                           adj_logits)
    res = run_bass_kernel_spmd(nc, in_maps, core_ids=list(range(N_CORES)))
    outs = [r["out"].reshape(B // N_CORES, S, E, D) for r in res.results]
    return np.concatenate(outs, axis=0)

